# revision 66
# baseline (speedup 1.0000x reference)
"""Low-rank attention kernel for Trainium2, distributed over 8 NeuronCores.

Math (per batch b):
    u  = q @ Wu            [N, R]
    vp = k @ Wv            [N, R]
    S  = u @ vp.T / sqrt(R)
    out = softmax(S) @ v   [N, D]

Shapes: B=4, N=4096, D=1024, R=32.

Sharding: data-parallel over batch x row-halves -> 8 shards. Core c handles
batch b = c // 2, rows [h*2048, (h+1)*2048) with h = c % 2. Each core gets its
q-shard and the full k/v for its batch, all in float16 (halves HBM traffic vs
f32; end-to-end max rel err ~1e-3 vs the 2e-2 budget).

Per-core device kernel (PE busy ~267us of ~285us CoreSim total; AV matmul
is 218us of it and runs at the f16 1-cycle/row roofline — fp8/DoubleRow was
measured numerically out of budget for the 2e-2 gate):
  1. uT[R, 2048]  = sum_d Wu[d, :].T qT[d, :]   (K=128 d-tiles, PSUM accum)
     vpT[R, 4096] = sum_d Wv[d, :].T kT[d, :]
     vp quarters and the late u chunks are interleaved into the flash stream
     so the PE never waits on the tail of the kT/qT DMA streams.
  2. one continuous flash pipeline over all (chunk, m-pair) steps:
       ps[m256-pair, n256] in one PSUM bank (two 128-col matmuls)
       ex = Exp(ps / sqrt(R))          one ScalarE instr per pair (f16 out)
       sum_acc[n128, 1]    += ex_tile.T @ ones      (issued before the AV
       out_acc[n128, d512] += ex_tile.T @ v_tile     matmuls so the final
                                                     reciprocal starts early)
     scores/exp for the next chunk are issued before the current chunk's AV
     tail, so chunk boundaries cost no exp-latency bubble.
     out = out_acc * (1 / sum_acc): recips on DVE, the [128,512] muls split
     across DVE + ScalarE (Copy shares the exp act-func table), o streamed
     out in f16 halves right behind each mul.

PSUM budget (8 banks): 4 AV accumulators + 2 paired-score banks + 1
projection accumulator + 1 row-sums bank.
"""

import numpy as np

B, N, D, R = 4, 4096, 1024, 32
NLOC = N // 2            # rows per core
RSCALE = float(1.0 / np.sqrt(np.float32(R)))

N_CHUNK = 256            # rows of scores computed per PSUM round
D_HALF = 512             # PSUM bank width in fp32

LAST_RESULT = None       # test.py reads exec_time_ns etc. from here
LAST_NC = None           # built Bass module, for test.py's bench loop
LAST_IN_MAPS = None      # per-core input maps, for test.py's bench loop


def _build():
    from concourse import bacc, mybir
    from concourse.tile import TileContext

    f32 = mybir.dt.float32
    f16 = mybir.dt.float16
    EXP = mybir.ActivationFunctionType.Exp

    nc = bacc.Bacc("TRN2", target_bir_lowering=False)

    qT = nc.dram_tensor("qT", [D, NLOC], f16, kind="ExternalInput")
    kT = nc.dram_tensor("kT", [D, N], f16, kind="ExternalInput")
    v = nc.dram_tensor("v", [N, D], f16, kind="ExternalInput")
    wu = nc.dram_tensor("wu", [D, R], f16, kind="ExternalInput")
    wv = nc.dram_tensor("wv", [D, R], f16, kind="ExternalInput")
    o = nc.dram_tensor("o", [NLOC, D], f16, kind="ExternalOutput")

    DT = D // 128         # 8 d-tiles
    NCH = NLOC // N_CHUNK  # 8 flash chunks
    MT = N // 128         # 32 m tiles
    NP = MT // 2          # 16 m-tile pairs per chunk
    GP = NCH * NP         # 128 global pairs
    VG = 16               # v row-groups of 256
    VPG = N // VG // 128  # 2 m-tiles per v group

    with TileContext(nc) as tc:
        with tc.tile_pool(name="singles", bufs=1) as singles, \
             tc.tile_pool(name="stream", bufs=12) as stream, \
             tc.tile_pool(name="vpool", bufs=VG) as vpool, \
             tc.tile_pool(name="expp", bufs=6) as expp, \
             tc.tile_pool(name="outp", bufs=6) as outp, \
             tc.tile_pool(name="rpool", bufs=4) as rpool, \
             tc.tile_pool(name="pacc", bufs=4, space="PSUM") as pacc, \
             tc.tile_pool(name="pscore", bufs=2, space="PSUM") as pscore, \
             tc.tile_pool(name="pproj", bufs=1, space="PSUM") as pproj, \
             tc.tile_pool(name="psums", bufs=1, space="PSUM") as psums:

            # ---- constants / projection weights ----
            # wv first: the prologue's first matmuls are the vp projection
            wv_sb = singles.tile([128, DT, R], f16, tag="wv")
            nc.sync.dma_start(out=wv_sb, in_=wv.rearrange("(t p) r -> p t r", p=128))
            wu_sb = singles.tile([128, DT, R], f16, tag="wu")
            nc.sync.dma_start(out=wu_sb, in_=wu.rearrange("(t p) r -> p t r", p=128))
            ones = singles.tile([128, 2], f16, tag="ones")
            nc.vector.memset(ones, 1.0)

            uT = singles.tile([R, NLOC], f16, tag="uT")
            vpT = singles.tile([R, N], f16, tag="vpT")

            # ---- DMA issue order = approximate arrival order ----
            # kT/qT loaded as [128, 8, 512] column-halves (one descriptor per
            # 512-col half across all 8 d-tiles: few, fat DMAs -> the SP
            # queue isn't descriptor-issue-bound). v groups interleaved in
            # the order the flash loop consumes them; qT h1 last (needed
            # from chunk 4, ~150us in).
            kt_tiles = {}

            def load_kt(qtr, c2, parts=1, eng=None):
                eng = eng or nc.sync
                tile = stream.tile([128, DT, 512], f16, tag="stream",
                                   name=f"kt{qtr}_{c2}")
                col = qtr * 1024 + c2 * 512
                dt2 = DT // parts
                for s in range(parts):
                    eng.dma_start(
                        out=tile[:, s * dt2:(s + 1) * dt2, :],
                        in_=kT[s * dt2 * 128:(s + 1) * dt2 * 128,
                               col:col + 512].rearrange(
                            "(t p) c -> p t c", p=128))
                kt_tiles[(qtr, c2)] = tile

            qt_tiles = {}

            def load_qt(h, c2, parts=1, eng=None):
                # eng: alternate HWDGE engine for the descriptor issue (the
                # first qT half rides the Activation queue so it doesn't
                # serialize behind kT q0 on the SP queue at kernel start)
                eng = eng or nc.sync
                tile = stream.tile([128, DT, 512], f16, tag="stream",
                                   name=f"qt{h}_{c2}")
                col = h * 1024 + c2 * 512
                dt2 = DT // parts
                for s in range(parts):
                    eng.dma_start(
                        out=tile[:, s * dt2:(s + 1) * dt2, :],
                        in_=qT[s * dt2 * 128:(s + 1) * dt2 * 128,
                               col:col + 512].rearrange(
                            "(t p) c -> p t c", p=128))
                qt_tiles[(h, c2)] = tile

            v_sb = [None] * VG

            def load_v(g, eng=None):
                eng = eng or nc.sync
                rows = VPG * 128
                vt = vpool.tile([128, VPG, D], f16, tag="v", name=f"v{g}")
                eng.dma_start(
                    out=vt, in_=v[g * rows:(g + 1) * rows, :].rearrange(
                        "(t p) d -> p t d", p=128))
                v_sb[g] = vt

            load_kt(0, 0, parts=4)
            load_kt(0, 1, parts=2, eng=nc.scalar)
            load_qt(0, 0, parts=2, eng=nc.scalar)
            load_qt(0, 1, parts=2, eng=nc.scalar)
            load_v(0, eng=nc.scalar)
            load_v(1, eng=nc.scalar)
            load_v(2)
            load_v(3)
            load_kt(1, 0)
            load_kt(1, 1)
            load_v(4)
            load_v(5)
            load_v(6)
            load_v(7)
            load_kt(2, 0)
            load_kt(2, 1)
            load_v(8)
            load_v(9)
            load_v(10)
            load_v(11)
            load_kt(3, 0)
            load_kt(3, 1)
            load_v(12)
            load_v(13)
            load_v(14)
            load_v(15)
            load_qt(1, 0)
            load_qt(1, 1)

            # dummy exp after the DMA issues (so the Activation engine's
            # descriptor issues aren't stuck behind the ones-memset dep):
            # forces the ScalarE act-func table DMA (~2.7us) to happen under
            # the input-DMA shadow, not on the first real exp
            warm = singles.tile([128, 2], f16, tag="warm")
            nc.scalar.activation(out=warm, in_=ones, func=EXP, scale=1.0)

            # ---- projection emitters (PE accum + DVE copy out of PSUM) ----
            def proj_512(w_sb, tiles, key, out_ap, name):
                pp = pproj.tile([R, 512], f32, tag="proj", name=name)
                for t in range(DT):
                    nc.tensor.matmul(pp, lhsT=w_sb[:, t, :],
                                     rhs=tiles[key][:, t, :],
                                     start=(t == 0), stop=(t == DT - 1))
                nc.vector.tensor_copy(out=out_ap, in_=pp)

            def u_chunk(c):
                h, c2 = c // 2, c % 2
                proj_512(wu_sb, qt_tiles, (h, c2),
                         uT[:, c * 512:(c + 1) * 512], f"pu{c}")

            def vp_half(qtr, c2):
                off = qtr * 1024 + c2 * 512
                proj_512(wv_sb, kt_tiles, (qtr, c2),
                         vpT[:, off:off + 512], f"pv{qtr}_{c2}")

            def vp_quarter(qtr):
                vp_half(qtr, 0)
                vp_half(qtr, 1)


            # ---- continuous flash pipeline over 128 global pairs ----
            # inject: global pair index -> thunk emitted before that pair's
            # scores are issued (slots projection work into the in-order PE
            # stream exactly where its inputs have arrived).
            inject = {2: lambda: vp_quarter(1),
                      6: lambda: vp_quarter(2),
                      10: lambda: vp_quarter(3),
                      34: lambda: u_chunk(2),
                      38: lambda: u_chunk(3)}

            def scores_exp(g):
                if g in inject:
                    inject[g]()
                ch = g // NP
                ps = pscore.tile([128, 2, N_CHUNK], f32, tag="scores",
                                 name=f"ps{g}")
                for i in range(2):
                    mt = 2 * (g % NP) + i
                    nc.tensor.matmul(
                        ps[:, i, :],
                        lhsT=vpT[:, mt * 128:(mt + 1) * 128],
                        rhs=uT[:, ch * N_CHUNK:(ch + 1) * N_CHUNK],
                        start=(i == 0), stop=(i == 1),
                        skip_group_check=True)
                ex = expp.tile([128, 2, N_CHUNK], f16, tag="ex", name=f"ex{g}")
                nc.scalar.activation(out=ex, in_=ps, func=EXP, scale=RSCALE)
                return ex

            accs = None
            sums = None

            def normalize(ch, accs, sums):
                jorder = (1, 0) if ch == NCH - 1 else (0, 1)
                # recips on DVE; [128,512] muls split DVE/ScalarE; the two
                # half-DMAs issue from different HWDGE queues (SP for the DVE
                # half, Activation for its own half) so output descriptors
                # don't serialize on one engine at the kernel tail.
                for j in jorder:
                    rc = rpool.tile([128, 1], f32, tag="rc", name=f"rc{ch}_{j}")
                    nc.vector.reciprocal(rc, sums[j][:, 0:1])
                    ob = outp.tile([128, D], f16, tag="ob", name=f"ob{ch}_{j}")
                    row = ch * N_CHUNK + j * 128
                    nc.vector.tensor_scalar_mul(ob[:, 0:D_HALF], accs[2 * j], rc)
                    nc.sync.dma_start(out=o[row:row + 128, 0:D_HALF],
                                      in_=ob[:, 0:D_HALF])
                    nc.scalar.mul(ob[:, D_HALF:D], accs[2 * j + 1], rc)
                    nc.scalar.dma_start(out=o[row:row + 128, D_HALF:D],
                                        in_=ob[:, D_HALF:D])

            # prologue: projections feeding chunk 0, then the pipeline
            vp_quarter(0)
            u_chunk(0)
            u_chunk(1)
            ex_q = [scores_exp(0), scores_exp(1)]
            for g in range(GP):
                ch, p = g // NP, g % NP
                if p == 0:
                    accs = [pacc.tile([128, D_HALF], f32, tag="acc",
                                      name=f"acc{ch}_{i}") for i in range(4)]
                    sums_t = psums.tile([128, 4], f32, tag="sums",
                                        name=f"sum{ch}")
                    sums = [sums_t[:, 0:2], sums_t[:, 2:4]]
                ex = ex_q.pop(0)
                if g + 2 < GP:
                    ex_q.append(scores_exp(g + 2))
                first_pair, last_pair = (p == 0), (p == NP - 1)
                for i in range(2):
                    mt = 2 * p + i
                    grp, tg = mt // VPG, mt % VPG
                    first, last = (first_pair and i == 0), \
                        (last_pair and i == 1)
                    for j in range(2):
                        lhs = ex[:, i, j * 128:(j + 1) * 128]
                        nc.tensor.matmul(sums[j], lhsT=lhs, rhs=ones,
                                         start=(first and j == 0), stop=last,
                                         skip_group_check=True)
                    halves = [(0, 0), (0, 1), (1, 0), (1, 1)]
                    if last and ch == NCH - 1:
                        # final group of the kernel: acc3 first so its
                        # normalize mul (queued first below) starts sooner
                        halves = halves[::-1]
                    for j, a in halves:
                        lhs = ex[:, i, j * 128:(j + 1) * 128]
                        rhs_v = v_sb[grp][:, tg, 0:D_HALF] if a == 0 \
                            else v_sb[grp][:, tg, D_HALF:D]
                        nc.tensor.matmul(accs[2 * j + a], lhsT=lhs,
                                         rhs=rhs_v, start=first, stop=last)
                if last_pair:
                    normalize(ch, accs, sums)

    nc.finalize()
    return nc


def kernel(q, k, v, Wu, Wv):
    global LAST_RESULT, LAST_NC, LAST_IN_MAPS
    from concourse import bass_utils

    nc = _build()
    LAST_NC = nc

    kTs = [np.ascontiguousarray(k[b].T.astype(np.float16)) for b in range(B)]
    vs = [np.ascontiguousarray(v[b]).astype(np.float16) for b in range(B)]
    wu16 = np.ascontiguousarray(Wu.astype(np.float16))
    wv16 = np.ascontiguousarray(Wv.astype(np.float16))
    in_maps = []
    for core in range(8):
        b, h = core // 2, core % 2
        in_maps.append({
            "qT": np.ascontiguousarray(
                q[b].T[:, h * NLOC:(h + 1) * NLOC].astype(np.float16)),
            "kT": kTs[b],
            "v": vs[b],
            "wu": wu16,
            "wv": wv16,
        })
    LAST_IN_MAPS = in_maps

    res = bass_utils.run_bass_kernel_spmd(nc, in_maps, core_ids=list(range(8)))
    LAST_RESULT = res

    out = np.empty((B, N, D), dtype=np.float32)
    for core in range(8):
        b, h = core // 2, core % 2
        out[b, h * NLOC:(h + 1) * NLOC, :] = \
            res.results[core]["o"].astype(np.float32)
    return out

